# revision 4
# baseline (speedup 1.0000x reference)
"""CavityLoss Trainium2 kernel (nn_CavityLoss_43722767073667), v2.

Mathematical reduction of the reference, exact in fp32 (verified against a
bit-faithful numpy emulation incl. adversarial threshold-boundary values):

  pb = (floor(pred*255) >= 128)  <=>  (pred >= c*),  c* = f32(128/255)
  diff = ((gt - pb*dilate(gt)) > 0) == gt * (1 - pb)          [identity]
  Non-critical voxels contribute exactly 0 to the BCE in fp32, so
      loss = -mean( gt * [pred < c*] * ln(pred) ).

v2 reformulation (one 1x DVE pass + one 2x DVE pass + two ACT passes,
instead of two 1x DVE passes + one ACT pass; DVE was the tail bottleneck):

    w  = p - gt                 (DVE tensor_tensor, 1 elem/cyc)
    crit <=> w < t1,  t1 = f32(c* - 1)       [single threshold merges both
        conditions: gt==1 gives w = p-1 in (-1,0), gt==0 gives w = p in (0,1);
        verified exhaustively: (f32(p-1) < t1) <=> (p < c*), and 1+t1 == c*]
    z  = min(w, t1)             (DVE tensor_scalar, 2x_2p mode, 2 elem/cyc)
    acc_sg[t] = sum Sign(t1 - w)         (ACT, accum; +1 crit / -1 non-crit)
    acc_ln[t] = sum Ln(z + 1)            (ACT, accum; crit -> ln(p),
                                          non-crit -> LN_C = Ln_table(c*))
    [Sign and Ln live in the same "natural_log" ACT table set -> one load,
     hoisted into the DMA window by a probe Ln that doubles as the LN_C
     calibration: probe = Ln_table(f32(t1+1)) on a memset tile, the exact
     same f32 input the non-critical pipeline path produces.]

  ones^T @ acc on the idle PE reduces partitions -> [1, 2NT+1], one 52-byte
  contiguous DMA out.  Host (f64):
     n_crit = (sum acc_sg + N) / 2
     loss = -(sum acc_ln - (N - n_crit) * LN_C) / N

Distribution: 192^3 volume flattened and split into 8 equal slabs (depth
sharding: 24 z-planes per core), each viewed as [128 partitions, 6912].
Pointwise + reduction only - the dilation cancels, so no halo exchange and
no collectives; the cross-core combine runs on the host in f64.

Scheduling notes:
  - pred/gt tiles stream on the single sync HWDGE ring, pred before gt per
    tile; only gt carries a semaphore - per-SDMA-engine FIFO within one ring
    means gt(t) complete implies pred(t) complete (one wait per instruction,
    TRN2 HW limit)
  - DVE per tile: tt (1x) then ts (2x) = ~3.0us/1728-tile, ACT per tile:
    Sign + Ln = ~3.2us/1728-tile, both under the 4.1us/tile DMA cadence
  - progressive tile sizes: small final tiles shorten the post-last-byte
    serial tail (tt+ts+Sign+Ln+matmul+copy+out-DMA)
"""

import numpy as np

import concourse.bacc as bacc
import concourse.mybir as mybir
from concourse.bass_utils import run_bass_kernel_spmd

D = 192
N_CORES = 8
P = 128
TOTAL = D * D * D              # 7_077_888
PER_CORE = TOTAL // N_CORES    # 884_736
FREE = PER_CORE // P           # 6_912
SIZES = [1728, 1728, 1728, 1248, 336, 144]
assert sum(SIZES) == FREE
NT = len(SIZES)
OUTW = 2 * NT + 1              # ln sums | sign sums | LN_C probe

C_STAR = float(np.float32(128.0) / np.float32(255.0))
T1 = float(np.float32(C_STAR - 1.0))   # -0.4980392...; f32(1+T1) == C_STAR

_CACHE = {}


def _build():
    nc = bacc.Bacc("TRN2", name="cavity_loss")
    f32 = mybir.dt.float32
    pred = nc.dram_tensor("pred", [P, FREE], f32, kind="ExternalInput")
    gt = nc.dram_tensor("gt", [P, FREE], f32, kind="ExternalInput")
    out = nc.dram_tensor("out", [1, OUTW], f32, kind="ExternalOutput")

    sub = mybir.AluOpType.subtract
    mn = mybir.AluOpType.min
    Ln = mybir.ActivationFunctionType.Ln
    Sign = mybir.ActivationFunctionType.Sign

    pred_sb = nc.alloc_sbuf_tensor("pred_sb", [P, FREE], f32).ap()
    gt_sb = nc.alloc_sbuf_tensor("gt_sb", [P, FREE], f32).ap()
    w_sb = nc.alloc_sbuf_tensor("w_sb", [P, FREE], f32).ap()
    z_sb = nc.alloc_sbuf_tensor("z_sb", [P, FREE], f32).ap()
    MAXW = max(SIZES)
    sgn_sb = nc.alloc_sbuf_tensor("sgn_sb", [P, MAXW], f32).ap()  # scratch
    ln_sb = nc.alloc_sbuf_tensor("ln_sb", [P, MAXW], f32).ap()    # scratch
    acc = nc.alloc_sbuf_tensor("acc_sb", [P, OUTW], f32).ap()
    t1_sb = nc.alloc_sbuf_tensor("t1_sb", [P, 1], f32).ap()
    probe_sb = nc.alloc_sbuf_tensor("probe_sb", [P, 1], f32).ap()

    s_gt = [nc.alloc_semaphore(f"s_gt{t}") for t in range(NT)]
    s_pred = nc.alloc_semaphore("s_pred")  # never waited; DGE needs sync info
    s_const = nc.alloc_semaphore("s_const")
    s_w = nc.alloc_semaphore("s_w")
    s_z = nc.alloc_semaphore("s_z")
    s_fin = nc.alloc_semaphore("s_fin")
    s_mm = nc.alloc_semaphore("s_mm")
    s_cp = nc.alloc_semaphore("s_cp")
    s_out = nc.alloc_semaphore("s_out")

    offs = np.concatenate([[0], np.cumsum(SIZES)]).tolist()
    sls = [slice(offs[t], offs[t + 1]) for t in range(NT)]

    # gpsimd: the T1 constant tile (Sign bias + LN_C probe input)
    nc.gpsimd.memset(t1_sb, T1).then_inc(s_const, 1)

    # sync: stream all tiles on one HWDGE ring, pred before gt per tile;
    # per-engine FIFO within the ring makes gt's sem cover pred too
    for t in range(NT):
        nc.sync.dma_start(pred_sb[:, sls[t]], pred[:, sls[t]]).then_inc(s_pred, 16)
        nc.sync.dma_start(gt_sb[:, sls[t]], gt[:, sls[t]]).then_inc(s_gt[t], 16)

    # vector: per tile, w = p - gt (1x) then z = min(w, T1) (2x_2p)
    for t in range(NT):
        sl = sls[t]
        nc.vector.wait_ge(s_gt[t], 16)
        nc.vector.tensor_tensor(
            w_sb[:, sl], pred_sb[:, sl], gt_sb[:, sl], sub
        ).then_inc(s_w, 1)
        nc.vector.tensor_scalar(z_sb[:, sl], w_sb[:, sl], T1, None, mn).then_inc(
            s_z, 1
        )

    # scalar: probe Ln first (pulls the natural_log ACT table load into the
    # DMA window AND calibrates LN_C = Ln_table(f32(T1+1)) == Ln_table(c*)),
    # then per tile Sign(T1 - w) and Ln(z + 1), each with a row-sum accum
    nc.scalar.wait_ge(s_const, 1)
    nc.scalar.activation(
        probe_sb[:], t1_sb, Ln, bias=1.0, scale=1.0,
        accum_out=acc[:, 2 * NT : 2 * NT + 1],
    )
    for t in range(NT):
        sl = sls[t]
        W = SIZES[t]
        nc.scalar.wait_ge(s_w, t + 1)
        nc.scalar.activation(
            sgn_sb[:, :W], w_sb[:, sl], Sign, bias=t1_sb, scale=-1.0,
            accum_out=acc[:, NT + t : NT + t + 1],
        )
        nc.scalar.wait_ge(s_z, t + 1)
        a = nc.scalar.activation(
            ln_sb[:, :W], z_sb[:, sl], Ln, bias=1.0, scale=1.0,
            accum_out=acc[:, t : t + 1],
        )
    a.then_inc(s_fin, 1)

    # finalize: partition-reduce acc on the (otherwise idle) TensorEngine,
    # then one contiguous tiny DMA: [1, OUTW] on one partition = 1 descriptor
    psum_fin = nc.alloc_psum_tensor("psum_fin", [1, OUTW], f32).ap()
    fin_sb = nc.alloc_sbuf_tensor("fin_sb", [1, OUTW], f32).ap()
    ones = nc.const_aps.tensor(1.0, (P, 1))
    nc.tensor.wait_ge(s_fin, 1)
    nc.tensor.matmul(psum_fin[:], ones, acc[:], start=True, stop=True).then_inc(
        s_mm, 1
    )
    nc.vector.wait_ge(s_mm, 1)
    nc.vector.tensor_copy(fin_sb[:], psum_fin[:]).then_inc(s_cp, 1)
    nc.sync.wait_ge(s_cp, 1)
    nc.sync.dma_start(out[:], fin_sb[:]).then_inc(s_out, 16)
    nc.sync.wait_ge(s_out, 16)

    nc.compile()
    return nc


def _get_nc():
    if "nc" not in _CACHE:
        _CACHE["nc"] = _build()
    return _CACHE["nc"]


def _shard(x):
    flat = np.ascontiguousarray(np.asarray(x, dtype=np.float32)).reshape(-1)
    assert flat.size == TOTAL, f"expected {TOTAL} elements, got {flat.size}"
    return [
        flat[c * PER_CORE : (c + 1) * PER_CORE].reshape(P, FREE)
        for c in range(N_CORES)
    ]


def run_spmd(pred, gt, **kw):
    """Shard, run on 8 cores; returns BassKernelResults (kw e.g. trace=True)."""
    preds = _shard(pred)
    gts = _shard(gt)
    in_maps = [{"pred": preds[c], "gt": gts[c]} for c in range(N_CORES)]
    return run_bass_kernel_spmd(
        _get_nc(), in_maps, core_ids=list(range(N_CORES)), **kw
    )


def kernel(pred, gt):
    res = run_spmd(pred, gt)
    ln_sum = 0.0
    sg_sum = 0.0
    ln_c = None
    for r in res.results:
        o = r["out"].astype(np.float64).reshape(OUTW)
        ln_sum += o[:NT].sum()
        sg_sum += o[NT : 2 * NT].sum()
        if ln_c is None:
            ln_c = o[2 * NT] / P  # probe col accumulated 1 value per partition
    n_crit = (sg_sum + TOTAL) / 2.0
    loss_sum = ln_sum - (TOTAL - n_crit) * ln_c
    return np.asarray(np.float32(-loss_sum / TOTAL))


# revision 5
# speedup vs baseline: 1.0293x; 1.0293x over previous
"""CavityLoss Trainium2 kernel (nn_CavityLoss_43722767073667), v3.

Mathematical reduction of the reference, exact in fp32 (verified against a
bit-faithful numpy emulation incl. adversarial threshold-boundary values):

  pb = (floor(pred*255) >= 128)  <=>  (pred >= c*),  c* = f32(128/255)
  diff = ((gt - pb*dilate(gt)) > 0) == gt * (1 - pb)          [identity]
  Non-critical voxels contribute exactly 0 to the BCE in fp32, so
      loss = -mean( gt * [pred < c*] * ln(pred) ).

Engine decomposition (one 1x + one 2x DVE pass + two ACT passes):

    w  = p - gt                 (DVE tensor_tensor, 1 elem/cyc)
    crit <=> w < t1,  t1 = f32(c* - 1)       [single threshold merges both
        conditions; verified exhaustively: (f32(p-1) < t1) <=> (p < c*),
        and f32(1+t1) == c*]
    z  = min(w, t1)             (DVE tensor_scalar, 2x_2p mode, 2 elem/cyc)
    acc_sg[t] = sum Sign(t1 - w)         (ACT accum; +1 crit / -1 non-crit)
    acc_ln[t] = sum Ln(z + 1)            (ACT accum; crit -> ln(p),
                                          non-crit -> LN_C = Ln_table(c*))
    [Sign and Ln share the "natural_log" ACT table set -> one table load,
     hoisted into the DMA window by a probe Ln that doubles as the LN_C
     calibration: Ln_table(f32(t1+1)), the exact non-critical-path input.]

  ones^T @ acc on the idle PE reduces partitions -> [1, 2NT+1], one 44-byte
  contiguous DMA out.  Host (f64):
     n_crit = (sum acc_sg + N) / 2
     loss = -(sum acc_ln - (N - n_crit) * LN_C) / N

Distribution: 192^3 volume flattened and split into 8 equal slabs, each
viewed as [128 partitions, 6912]. Pointwise + reduction only - the dilation
cancels, so no halo exchange and no collectives; cross-core combine on the
host in f64.

Scheduling notes (v3, from HW trace analysis of v2):
  - DMA completion semaphores release 1.5-5us AFTER the last data byte,
    the lag growing with the number of in-flight transfers on a ring.
    So: pred tiles stream on the sync HWDGE ring, gt tiles on the scalar
    HWDGE ring (5 transfers each instead of 10/12 on one ring); SDMA
    engines round-robin between rings at packet granularity so pair t
    completes at the same wall time, but each ring's completion chain is
    half as deep.
  - tile sizes [960, 1920x3, 192]: small first tile -> first semaphore
    releases ~2us earlier (pipeline spin-up); tiny last tile -> short
    post-last-release serial tail (tt+min+Sign+Ln+matmul+copy+out-DMA).
  - DVE takes two waits (pred + gt) as separate wait instructions (TRN2
    allows one wait per instruction).
"""

import numpy as np

import concourse.bacc as bacc
import concourse.mybir as mybir
from concourse.bass_utils import run_bass_kernel_spmd

D = 192
N_CORES = 8
P = 128
TOTAL = D * D * D              # 7_077_888
PER_CORE = TOTAL // N_CORES    # 884_736
FREE = PER_CORE // P           # 6_912
SIZES = [960, 1920, 1920, 1920, 192]
assert sum(SIZES) == FREE
NT = len(SIZES)
OUTW = 2 * NT + 1              # ln sums | sign sums | LN_C probe

C_STAR = float(np.float32(128.0) / np.float32(255.0))
T1 = float(np.float32(C_STAR - 1.0))   # -0.4980392...; f32(1+T1) == C_STAR

_CACHE = {}


def _build():
    nc = bacc.Bacc("TRN2", name="cavity_loss")
    f32 = mybir.dt.float32
    pred = nc.dram_tensor("pred", [P, FREE], f32, kind="ExternalInput")
    gt = nc.dram_tensor("gt", [P, FREE], f32, kind="ExternalInput")
    out = nc.dram_tensor("out", [1, OUTW], f32, kind="ExternalOutput")

    sub = mybir.AluOpType.subtract
    mn = mybir.AluOpType.min
    Ln = mybir.ActivationFunctionType.Ln
    Sign = mybir.ActivationFunctionType.Sign

    pred_sb = nc.alloc_sbuf_tensor("pred_sb", [P, FREE], f32).ap()
    gt_sb = nc.alloc_sbuf_tensor("gt_sb", [P, FREE], f32).ap()
    w_sb = nc.alloc_sbuf_tensor("w_sb", [P, FREE], f32).ap()
    z_sb = nc.alloc_sbuf_tensor("z_sb", [P, FREE], f32).ap()
    MAXW = max(SIZES)
    sgn_sb = nc.alloc_sbuf_tensor("sgn_sb", [P, MAXW], f32).ap()  # scratch
    ln_sb = nc.alloc_sbuf_tensor("ln_sb", [P, MAXW], f32).ap()    # scratch
    acc = nc.alloc_sbuf_tensor("acc_sb", [P, OUTW], f32).ap()
    t1_sb = nc.alloc_sbuf_tensor("t1_sb", [P, 1], f32).ap()
    probe_sb = nc.alloc_sbuf_tensor("probe_sb", [P, 1], f32).ap()

    s_pred = [nc.alloc_semaphore(f"s_pred{t}") for t in range(NT)]
    s_gt = [nc.alloc_semaphore(f"s_gt{t}") for t in range(NT)]
    s_const = nc.alloc_semaphore("s_const")
    s_w = nc.alloc_semaphore("s_w")
    s_z = nc.alloc_semaphore("s_z")
    s_fin = nc.alloc_semaphore("s_fin")
    s_mm = nc.alloc_semaphore("s_mm")
    s_cp = nc.alloc_semaphore("s_cp")
    s_out = nc.alloc_semaphore("s_out")

    offs = np.concatenate([[0], np.cumsum(SIZES)]).tolist()
    sls = [slice(offs[t], offs[t + 1]) for t in range(NT)]

    # gpsimd: the T1 constant tile (Sign bias + LN_C probe input)
    nc.gpsimd.memset(t1_sb, T1).then_inc(s_const, 1)

    # two independent HWDGE rings: pred on sync, gt on scalar
    for t in range(NT):
        nc.sync.dma_start(pred_sb[:, sls[t]], pred[:, sls[t]]).then_inc(
            s_pred[t], 16
        )
    for t in range(NT):
        nc.scalar.dma_start(gt_sb[:, sls[t]], gt[:, sls[t]]).then_inc(s_gt[t], 16)

    # vector: per tile, w = p - gt (1x) then z = min(w, T1) (2x_2p)
    for t in range(NT):
        sl = sls[t]
        nc.vector.wait_ge(s_pred[t], 16)
        nc.vector.wait_ge(s_gt[t], 16)
        nc.vector.tensor_tensor(
            w_sb[:, sl], pred_sb[:, sl], gt_sb[:, sl], sub
        ).then_inc(s_w, 1)
        nc.vector.tensor_scalar(z_sb[:, sl], w_sb[:, sl], T1, None, mn).then_inc(
            s_z, 1
        )

    # scalar: probe Ln first (pulls the natural_log ACT table load into the
    # DMA window AND calibrates LN_C = Ln_table(f32(T1+1)) == Ln_table(c*)),
    # then per tile Sign(T1 - w) and Ln(z + 1), each with a row-sum accum
    nc.scalar.wait_ge(s_const, 1)
    nc.scalar.activation(
        probe_sb[:], t1_sb, Ln, bias=1.0, scale=1.0,
        accum_out=acc[:, 2 * NT : 2 * NT + 1],
    )
    for t in range(NT):
        sl = sls[t]
        W = SIZES[t]
        nc.scalar.wait_ge(s_w, t + 1)
        nc.scalar.activation(
            sgn_sb[:, :W], w_sb[:, sl], Sign, bias=t1_sb, scale=-1.0,
            accum_out=acc[:, NT + t : NT + t + 1],
        )
        nc.scalar.wait_ge(s_z, t + 1)
        a = nc.scalar.activation(
            ln_sb[:, :W], z_sb[:, sl], Ln, bias=1.0, scale=1.0,
            accum_out=acc[:, t : t + 1],
        )
    a.then_inc(s_fin, 1)

    # finalize: partition-reduce acc on the (otherwise idle) TensorEngine,
    # then one contiguous tiny DMA: [1, OUTW] on one partition = 1 descriptor
    psum_fin = nc.alloc_psum_tensor("psum_fin", [1, OUTW], f32).ap()
    fin_sb = nc.alloc_sbuf_tensor("fin_sb", [1, OUTW], f32).ap()
    ones = nc.const_aps.tensor(1.0, (P, 1))
    nc.tensor.wait_ge(s_fin, 1)
    nc.tensor.matmul(psum_fin[:], ones, acc[:], start=True, stop=True).then_inc(
        s_mm, 1
    )
    nc.vector.wait_ge(s_mm, 1)
    nc.vector.tensor_copy(fin_sb[:], psum_fin[:]).then_inc(s_cp, 1)
    nc.sync.wait_ge(s_cp, 1)
    nc.sync.dma_start(out[:], fin_sb[:]).then_inc(s_out, 16)
    nc.sync.wait_ge(s_out, 16)

    nc.compile()
    return nc


def _get_nc():
    if "nc" not in _CACHE:
        _CACHE["nc"] = _build()
    return _CACHE["nc"]


def _shard(x):
    flat = np.ascontiguousarray(np.asarray(x, dtype=np.float32)).reshape(-1)
    assert flat.size == TOTAL, f"expected {TOTAL} elements, got {flat.size}"
    return [
        flat[c * PER_CORE : (c + 1) * PER_CORE].reshape(P, FREE)
        for c in range(N_CORES)
    ]


def run_spmd(pred, gt, **kw):
    """Shard, run on 8 cores; returns BassKernelResults (kw e.g. trace=True)."""
    preds = _shard(pred)
    gts = _shard(gt)
    in_maps = [{"pred": preds[c], "gt": gts[c]} for c in range(N_CORES)]
    return run_bass_kernel_spmd(
        _get_nc(), in_maps, core_ids=list(range(N_CORES)), **kw
    )


def kernel(pred, gt):
    res = run_spmd(pred, gt)
    ln_sum = 0.0
    sg_sum = 0.0
    ln_c = None
    for r in res.results:
        o = r["out"].astype(np.float64).reshape(OUTW)
        ln_sum += o[:NT].sum()
        sg_sum += o[NT : 2 * NT].sum()
        if ln_c is None:
            ln_c = o[2 * NT] / P  # probe col accumulated once per partition
    n_crit = (sg_sum + TOTAL) / 2.0
    loss_sum = ln_sum - (TOTAL - n_crit) * ln_c
    return np.asarray(np.float32(-loss_sum / TOTAL))


# revision 9
# speedup vs baseline: 1.1479x; 1.1152x over previous
"""CavityLoss Trainium2 kernel (nn_CavityLoss_43722767073667).

Mathematical reduction of the reference, exact in fp32 (verified against a
bit-faithful numpy emulation incl. adversarial threshold-boundary values):

  pb = (floor(pred*255) >= 128)  <=>  (pred >= c*),  c* = f32(128/255)
  The 5^3 all-ones dilation of the binary gt is an exact integer count
  >= gt (the window contains the center voxel), so
      diff = ((gt - pb*dilate(gt)) > 0) == gt * (1 - pb)     [identity]
  Non-critical voxels contribute exactly 0 to the BCE in fp32:
      clip(0, 1e-12, 1-1e-12) -> 1e-12, and fp32(1 - 1e-12) == 1.0,
      so (1-lc)*log(1-pc_c) == log(1.0) == 0.
  Therefore  loss = -mean( gt * [pred < c*] * ln(pred) ).

Distribution: 192^3 volume flattened and split into 8 equal slabs (depth
sharding: 24 z-planes per core), each viewed as [128 partitions, 6912].
Pointwise + reduction only - the dilation cancels, so no halo exchange and
no collectives; the cross-core mean is combined on the host in f64.

Per-core device kernel (raw bacc, hand-rolled semaphores, no Tile):
  sync engine streams pred/gt tiles in on the qSP HWDGE ring
  DVE   STT#1: r = (p is_ge c*) max p        # r = p where p<c*, else 1.0
  ACT   Ln:    l = ln(r)                     # masked ln; ln(1) ~ 0
  DVE   STT#2: (l bypass 1) mult gt, accum_out -> per-partition row sums
  PE    ones^T @ acc                         # 128-partition reduce -> [1,NT]
  sync  one contiguous 20-byte DMA of the [1,NT] result

Scheduling notes (measured on HW):
  - one semaphore per DMA transfer (completion order across queues is not
    FIFO, a shared counter would race - caught by CoreSim)
  - exactly one wait per instruction (TRN2 HW limit; gt arrival is proxied
    through ACT's wait so DVE's STT#2 only waits on s_l)
  - DVE stream is software-pipelined (STT#1(t+1) before STT#2(t)) so the
    serial STT#1->Ln->STT#2 chain spans tiles instead of serializing
  - a dummy Ln on the const-1.0 tile hoists the ~2.7us ACT_TABLE_LOAD
    into the DMA wait window
  - progressive tile sizes: the last tile is small so the post-last-byte
    compute tail (Ln + STT#2 of the final tile) is short
"""

import numpy as np

import concourse.bacc as bacc
import concourse.mybir as mybir
from concourse.bass_utils import run_bass_kernel_spmd

D = 192
N_CORES = 8
P = 128
TOTAL = D * D * D              # 7_077_888
PER_CORE = TOTAL // N_CORES    # 884_736
FREE = PER_CORE // P           # 6_912
SIZES = [1728, 1728, 1728, 1152, 576]
assert sum(SIZES) == FREE
NT = len(SIZES)

C_STAR = float(np.float32(128.0) / np.float32(255.0))

_CACHE = {}


def _build():
    nc = bacc.Bacc("TRN2", name="cavity_loss")
    f32 = mybir.dt.float32
    pred = nc.dram_tensor("pred", [P, FREE], f32, kind="ExternalInput")
    gt = nc.dram_tensor("gt", [P, FREE], f32, kind="ExternalInput")
    out = nc.dram_tensor("out", [1, NT], f32, kind="ExternalOutput")

    ge = mybir.AluOpType.is_ge
    mx = mybir.AluOpType.max
    byp = mybir.AluOpType.bypass
    mul = mybir.AluOpType.mult
    Ln = mybir.ActivationFunctionType.Ln

    pred_sb = nc.alloc_sbuf_tensor("pred_sb", [P, FREE], f32).ap()
    gt_sb = nc.alloc_sbuf_tensor("gt_sb", [P, FREE], f32).ap()
    r_sb = nc.alloc_sbuf_tensor("r_sb", [P, FREE], f32).ap()
    l_sb = nc.alloc_sbuf_tensor("l_sb", [P, FREE], f32).ap()
    acc = nc.alloc_sbuf_tensor("acc_sb", [P, NT], f32).ap()

    s_pred = [nc.alloc_semaphore(f"s_pred{t}") for t in range(NT)]
    s_gt = [nc.alloc_semaphore(f"s_gt{t}") for t in range(NT)]
    s_r = nc.alloc_semaphore("s_r")
    s_l = nc.alloc_semaphore("s_l")
    s_acc = nc.alloc_semaphore("s_acc")
    s_mm = nc.alloc_semaphore("s_mm")
    s_fin = nc.alloc_semaphore("s_fin")
    s_out = nc.alloc_semaphore("s_out")

    offs = np.concatenate([[0], np.cumsum(SIZES)]).tolist()
    sls = [slice(offs[t], offs[t + 1]) for t in range(NT)]

    # sync: stream all tiles in on one HWDGE ring, pred before gt per tile
    for t in range(NT):
        nc.sync.dma_start(pred_sb[:, sls[t]], pred[:, sls[t]]).then_inc(s_pred[t], 16)
        nc.sync.dma_start(gt_sb[:, sls[t]], gt[:, sls[t]]).then_inc(s_gt[t], 16)

    # scalar: dummy Ln pulls ACT_TABLE_LOAD into the DMA window, then the
    # per-tile Ln chain (gt arrival proxied so STT#2 needs a single wait)
    dummy = nc.alloc_sbuf_tensor("dummy_sb", [P, 1], f32).ap()
    nc.scalar.activation(dummy[:], nc.const_aps.tensor(1.0, (P, 1)), Ln)
    for t in range(NT):
        sl = sls[t]
        nc.scalar.wait_ge(s_gt[t], 16)
        nc.scalar.wait_ge(s_r, t + 1)
        nc.scalar.activation(l_sb[:, sl], r_sb[:, sl], Ln).then_inc(s_l, 1)

    # vector, software-pipelined across tiles
    def stt1(t):
        sl = sls[t]
        nc.vector.wait_ge(s_pred[t], 16)
        nc.vector.scalar_tensor_tensor(
            r_sb[:, sl], pred_sb[:, sl], C_STAR, pred_sb[:, sl], ge, mx
        ).then_inc(s_r, 1)

    def stt2(t):
        sl = sls[t]
        nc.vector.wait_ge(s_l, t + 1)
        # out lands over r_sb tile t: dead after Ln(t), ordered via s_l wait
        nc.vector.scalar_tensor_tensor(
            r_sb[:, sl], l_sb[:, sl], 1.0, gt_sb[:, sl], byp, mul,
            accum_out=acc[:, t : t + 1],
        ).then_inc(s_acc, 1)

    stt1(0)
    for t in range(1, NT):
        stt1(t)
        stt2(t - 1)
    stt2(NT - 1)

    # finalize: partition-reduce acc on the (otherwise idle) TensorEngine,
    # then one contiguous tiny DMA: [1, NT] on one partition = 1 descriptor
    psum_fin = nc.alloc_psum_tensor("psum_fin", [1, NT], f32).ap()
    fin_sb = nc.alloc_sbuf_tensor("fin_sb", [1, NT], f32).ap()
    ones = nc.const_aps.tensor(1.0, (P, 1))
    nc.tensor.wait_ge(s_acc, NT)
    nc.tensor.matmul(
        psum_fin[:], ones, acc[:], start=True, stop=True
    ).then_inc(s_mm, 1)
    nc.vector.wait_ge(s_mm, 1)
    nc.vector.tensor_copy(fin_sb[:], psum_fin[:]).then_inc(s_fin, 1)
    nc.sync.wait_ge(s_fin, 1)
    nc.sync.dma_start(out[:], fin_sb[:]).then_inc(s_out, 16)
    nc.sync.wait_ge(s_out, 16)

    nc.compile()
    return nc


def _get_nc():
    if "nc" not in _CACHE:
        _CACHE["nc"] = _build()
    return _CACHE["nc"]


def _shard(x):
    flat = np.ascontiguousarray(np.asarray(x, dtype=np.float32)).reshape(-1)
    assert flat.size == TOTAL, f"expected {TOTAL} elements, got {flat.size}"
    return [
        flat[c * PER_CORE : (c + 1) * PER_CORE].reshape(P, FREE)
        for c in range(N_CORES)
    ]


def run_spmd(pred, gt, **kw):
    """Shard, run on 8 cores; returns BassKernelResults (kw e.g. trace=True)."""
    preds = _shard(pred)
    gts = _shard(gt)
    in_maps = [{"pred": preds[c], "gt": gts[c]} for c in range(N_CORES)]
    return run_bass_kernel_spmd(
        _get_nc(), in_maps, core_ids=list(range(N_CORES)), **kw
    )


def kernel(pred, gt):
    res = run_spmd(pred, gt)
    total = 0.0
    for r in res.results:
        total += float(r["out"].astype(np.float64).sum())
    return np.asarray(np.float32(-total / TOTAL))

